# revision 2
# baseline (speedup 1.0000x reference)
"""Trainium2 Bass kernel for CirculatePairConLoss.

Reference math (N=4096, D=64, C=16, T=0.05):
    feats = concat(f1, f2)                  # [2N, D]
    sim   = exp(feats @ feats.T / T)        # [2N, 2N]
    Ng_i  = sum_{j: lab_j != lab_i} sim_ij
    pos_i = exp(<f1_i, f2_i> / T)           (duplicated for both halves)
    term  = -log(pos / (Ng + pos))
    loss  = sum(term / group_size),  group_size_i = 2 * count(label == lab_i)

Device strategy (8 cores, SPMD, full I/O):
  Each core owns a 1024-wide slice of the 8192 "i" columns.  For each
  128-row j-subtile it computes logits = F_j^T F_i on the tensor engine
  (bf16, K=64), exponentiates on the scalar engine (batched 1536 wide,
  PSUM -> SBUF bf16, scale=1/T folded in), then contracts the exp'd tile
  against a one-hot label matrix H [128j, 16c] on the tensor engine,
  accumulating per-class sums acc[c, i] in fp32 PSUM.  That yields, for
  every i, the per-class row sums of sim -- from which the host derives
  tot_i (sum over classes) and same_i (class of i) in a trivial epilogue.
  The f1.f2 dots are computed on-device via an elementwise multiply and a
  ones-vector fp32 matmul.  Per-core outputs are tiny ([16,1024] class
  sums + [1,1024] dots); the host gather + log epilogue is O(2N*C).
"""

import numpy as np
import ml_dtypes

import concourse.bass as bass
import concourse.tile as tile
from concourse import bacc, mybir
from concourse.bass_utils import run_bass_kernel_spmd

N = 4096
D = 64
C = 16
TWO_N = 2 * N
TEMP = 0.05
SCALE = 1.0 / TEMP  # 20.0
NCORES = 8

IBLK = 512          # i-block width (one PSUM bank of fp32 per matmul)
JSUB = 128          # j-subtile height (matmul partition dim of the H contraction)
I_PER_CORE = TWO_N // NCORES          # 1024
NB_I_LOCAL = I_PER_CORE // IBLK       # 2 i-blocks per core
NB_J = TWO_N // JSUB                  # 64 j-subtiles
ACT_GROUP = 3                         # j-subtiles exp'd per ScalarE instruction

BF16 = mybir.dt.bfloat16
F32 = mybir.dt.float32

_CACHE = {}


def _build_program():
    """Build the (core-uniform) Bass program once."""
    nc = bacc.Bacc("TRN2", target_bir_lowering=False, debug=False,
                   num_devices=NCORES)

    # ---- I/O ----
    ft_all = nc.declare_dram_parameter("ft_all", [D, TWO_N], BF16, isOutput=False)
    ft_i = nc.declare_dram_parameter("ft_i", [D, I_PER_CORE], BF16, isOutput=False)
    h_all = nc.declare_dram_parameter("h_all", [JSUB, NB_J * C], BF16, isOutput=False)
    a_i = nc.declare_dram_parameter("a_i", [D, I_PER_CORE], F32, isOutput=False)
    b_i = nc.declare_dram_parameter("b_i", [D, I_PER_CORE], F32, isOutput=False)
    acc_out = nc.declare_dram_parameter("acc_out", [C, I_PER_CORE], F32, isOutput=True)
    dots_out = nc.declare_dram_parameter("dots_out", [1, I_PER_CORE], F32, isOutput=True)

    # subtile stream: for each local i-block, all 64 j-subtiles
    stream = [(ib, js) for ib in range(NB_I_LOCAL) for js in range(NB_J)]
    n_sub = len(stream)

    with tile.TileContext(nc) as tc:
        with (
            tc.tile_pool(name="consts", bufs=1) as consts,
            tc.tile_pool(name="simpool", bufs=3) as simpool,
            tc.tile_pool(name="small", bufs=2) as small,
            tc.tile_pool(name="plog", bufs=2, space="PSUM") as plog,
            tc.tile_pool(name="pacc", bufs=2, space="PSUM") as pacc,
        ):
            ft_all_sb = consts.tile([D, TWO_N], BF16)
            nc.sync.dma_start(out=ft_all_sb, in_=ft_all[:])
            ft_i_sb = consts.tile([D, I_PER_CORE], BF16)
            nc.sync.dma_start(out=ft_i_sb, in_=ft_i[:])
            h_sb = consts.tile([JSUB, NB_J * C], BF16)
            nc.sync.dma_start(out=h_sb, in_=h_all[:])
            a_sb = consts.tile([D, I_PER_CORE], F32)
            nc.sync.dma_start(out=a_sb, in_=a_i[:])
            b_sb = consts.tile([D, I_PER_CORE], F32)
            nc.sync.dma_start(out=b_sb, in_=b_i[:])
            ones_sb = consts.tile([D, 1], F32)
            nc.vector.memset(ones_sb, 1.0)

            acc_sb = consts.tile([C, I_PER_CORE], F32)
            dots_sb = consts.tile([1, I_PER_CORE], F32)

            acc_ps = None
            # iterate ACT groups over the subtile stream
            g0 = 0
            while g0 < n_sub:
                gsz = min(ACT_GROUP, n_sub - g0)
                # never let a group straddle an i-block boundary mid-acc:
                # (group can straddle: H matmuls are per-subtile; acc groups
                # open/close on i-block boundaries below)
                lg = plog.tile([JSUB, ACT_GROUP * IBLK], F32, tag="lg")
                for u in range(gsz):
                    ib, js = stream[g0 + u]
                    nc.tensor.matmul(
                        lg[:, u * IBLK:(u + 1) * IBLK],
                        ft_all_sb[:, js * JSUB:(js + 1) * JSUB],
                        ft_i_sb[:, ib * IBLK:(ib + 1) * IBLK],
                        start=True, stop=True,
                    )
                sim = simpool.tile([JSUB, ACT_GROUP * IBLK], BF16, tag="sim")
                nc.scalar.activation(
                    out=sim[:, :gsz * IBLK],
                    in_=lg[:, :gsz * IBLK],
                    func=mybir.ActivationFunctionType.Exp,
                    scale=SCALE,
                )
                for u in range(gsz):
                    ib, js = stream[g0 + u]
                    if js == 0:
                        acc_ps = pacc.tile([C, IBLK], F32, tag="acc")
                    nc.tensor.matmul(
                        acc_ps,
                        h_sb[:, js * C:(js + 1) * C],
                        sim[:, u * IBLK:(u + 1) * IBLK],
                        start=(js == 0), stop=(js == NB_J - 1),
                    )
                    if js == NB_J - 1:
                        nc.vector.tensor_copy(
                            acc_sb[:, ib * IBLK:(ib + 1) * IBLK], acc_ps)
                g0 += gsz

            # f1.f2 dots: prod = a*b (fp32, DVE), then ones^T @ prod on PE
            for ib in range(NB_I_LOCAL):
                sl = slice(ib * IBLK, (ib + 1) * IBLK)
                prod = small.tile([D, IBLK], F32, tag="prod")
                nc.vector.tensor_mul(prod, a_sb[:, sl], b_sb[:, sl])
                dps = pacc.tile([1, IBLK], F32, tag="acc")
                nc.tensor.matmul(dps, ones_sb, prod, start=True, stop=True)
                nc.vector.tensor_copy(dots_sb[:, sl], dps)

            nc.sync.dma_start(out=acc_out[:], in_=acc_sb)
            nc.sync.dma_start(out=dots_out[:], in_=dots_sb)

    nc.compile()
    return nc


def kernel(f1, f2, label):
    f1 = np.asarray(f1, dtype=np.float32)
    f2 = np.asarray(f2, dtype=np.float32)
    label = np.asarray(label).astype(np.int64)

    if "nc" not in _CACHE:
        _CACHE["nc"] = _build_program()
    nc = _CACHE["nc"]

    feats = np.concatenate([f1, f2], axis=0)              # [2N, D]
    lab2 = np.concatenate([label, label], axis=0)         # [2N]
    ft = np.ascontiguousarray(feats.T)                    # [D, 2N] f32
    ft_bf = ft.astype(ml_dtypes.bfloat16)
    f1t = np.ascontiguousarray(f1.T).astype(np.float32)   # [D, N]
    f2t = np.ascontiguousarray(f2.T).astype(np.float32)

    # one-hot per j-subtile, packed [128, 64*16]
    h_pack = np.zeros((JSUB, NB_J * C), dtype=ml_dtypes.bfloat16)
    eye = np.eye(C, dtype=np.float32)
    for js in range(NB_J):
        rows = lab2[js * JSUB:(js + 1) * JSUB]
        h_pack[:, js * C:(js + 1) * C] = eye[rows].astype(ml_dtypes.bfloat16)

    in_maps = []
    for k in range(NCORES):
        isl = slice(k * I_PER_CORE, (k + 1) * I_PER_CORE)
        r0 = (k * I_PER_CORE) % N
        rsl = slice(r0, r0 + I_PER_CORE)
        in_maps.append({
            "ft_all": ft_bf,
            "ft_i": np.ascontiguousarray(ft_bf[:, isl]),
            "h_all": h_pack,
            "a_i": np.ascontiguousarray(f1t[:, rsl]),
            "b_i": np.ascontiguousarray(f2t[:, rsl]),
        })

    res = run_bass_kernel_spmd(nc, in_maps, core_ids=list(range(NCORES)))
    _CACHE["last_res"] = res  # lets a test harness read exec_time_ns after tracing

    # ---- host epilogue (tiny): combine per-core class sums ----
    acc = np.zeros((C, TWO_N), dtype=np.float64)
    dots = np.zeros(TWO_N, dtype=np.float64)
    for k in range(NCORES):
        isl = slice(k * I_PER_CORE, (k + 1) * I_PER_CORE)
        acc[:, isl] = res.results[k]["acc_out"].astype(np.float64)
        dots[isl] = res.results[k]["dots_out"][0].astype(np.float64)

    tot = acc.sum(axis=0)                                  # [2N]
    same = acc[lab2, np.arange(TWO_N)]                     # [2N]
    ng = tot - same
    logpos = SCALE * dots
    pos = np.exp(logpos)
    term = np.log(ng + pos) - logpos
    counts = np.bincount(label, minlength=C)
    group_size = 2.0 * counts[lab2]
    loss = np.sum(term / group_size)
    return np.float32(loss)


# revision 4
# speedup vs baseline: 1.0912x; 1.0912x over previous
"""Trainium2 Bass kernel for CirculatePairConLoss.

Reference math (N=4096, D=64, C=16, T=0.05):
    feats = concat(f1, f2)                  # [2N, D]
    sim   = exp(feats @ feats.T / T)        # [2N, 2N]
    Ng_i  = sum_{j: lab_j != lab_i} sim_ij
    pos_i = exp(<f1_i, f2_i> / T)           (duplicated for both halves)
    term  = -log(pos / (Ng + pos))
    loss  = sum(term / group_size),  group_size_i = 2 * count(label == lab_i)

Device strategy (8 cores, SPMD, full I/O), symmetric version:
  Rows are sorted by label so each class is a contiguous run.  The [16x16]
  grid of 512-wide blocks of the symmetric sim matrix is covered by its
  upper triangle (136 blocks); core k owns block-rows {k, 15-k} (17 blocks
  each -- perfectly balanced).  For each block (r, c):
    * logits = Fs_r^T Fs_c on the tensor engine (bf16, K=64),
    * exp on the scalar engine (batched 1536 wide, scale=1/T folded in),
    * free side (index y in c): per-class sums via a one-hot H-matmul on
      the tensor engine, fp32 PSUM accumulation over the block,
    * partition side (index x in r): row totals via a segmented DVE
      reduce; same-class windows exist only for c in {r, r+1, r+2}
      (classes span <= 3 blocks) -- diag is covered by the H-matmul and
      the near-diag blocks get a masked fused multiply-reduce on DVE.
  Per-core outputs are tiny (class sums / totals / masked sums / f1.f2
  dots); the host does an O(2N*C) gather + log epilogue.

Every core runs the identical program; all per-core differences live in
packed input tensors (gathered slices of the sorted feature matrix, the
one-hot blocks, and the near-diagonal equality masks).
"""

import numpy as np
import ml_dtypes

import concourse.bass as bass
import concourse.tile as tile
from concourse import bacc, mybir
from concourse.bass_utils import run_bass_kernel_spmd

N = 4096
D = 64
C = 16
TWO_N = 2 * N
TEMP = 0.05
SCALE = 1.0 / TEMP  # 20.0
NCORES = 8

BLK = 512                      # block width (i and j)
JSUB = 128                     # partition-side subtile
NBLK = TWO_N // BLK            # 16
NBLOCKS = 17                   # upper-triangle blocks per core
NSUB = NBLOCKS * 4             # 68 subtiles per core
ACT_GROUP = 3                  # subtiles exp'd per ScalarE instruction
MASK_SLOTS = [13, 14, 15, 16]  # block positions that get equality masks
N_MASKED = len(MASK_SLOTS) * 4
I_PER_CORE = TWO_N // NCORES   # 1024 (for the f1.f2 dots)

BF16 = mybir.dt.bfloat16
F32 = mybir.dt.float32

_CACHE = {}


def _core_blocks(k):
    """Core k's 17 upper-triangle blocks: diag first, near-diag (mask
    candidates) in the fixed tail slots MASK_SLOTS."""
    r1, r2 = k, NBLK - 1 - k
    blocks = [(r1, c) for c in range(r1, NBLK)] + \
             [(r2, c) for c in range(r2, NBLK)]

    def prio(b):
        r, c = b
        if c == r:
            cls = 0           # diagonal
        elif c <= r + 2:
            cls = 2           # near-diagonal (needs mask)
        else:
            cls = 1           # far
        return (cls, r * NBLK + c)

    out = sorted(blocks, key=prio)
    assert len(out) == NBLOCKS
    return out


# ---------------------------------------------------------------------------
# symmetric (v2) program
# ---------------------------------------------------------------------------

def _build_v2():
    nc = bacc.Bacc("TRN2", target_bir_lowering=False, debug=False,
                   num_devices=NCORES)

    ftl = nc.declare_dram_parameter("ftl", [D, NSUB * JSUB], BF16, isOutput=False)
    ftr = nc.declare_dram_parameter("ftr", [D, NBLOCKS * BLK], BF16, isOutput=False)
    hx = nc.declare_dram_parameter("hx", [JSUB, NSUB * C], BF16, isOutput=False)
    msk = nc.declare_dram_parameter("msk", [JSUB, N_MASKED * BLK], BF16, isOutput=False)
    a_i = nc.declare_dram_parameter("a_i", [D, I_PER_CORE], F32, isOutput=False)
    b_i = nc.declare_dram_parameter("b_i", [D, I_PER_CORE], F32, isOutput=False)

    acc_out = nc.declare_dram_parameter("acc_out", [C, NBLOCKS * BLK], F32, isOutput=True)
    tot_out = nc.declare_dram_parameter("tot_out", [JSUB, NSUB], F32, isOutput=True)
    same_out = nc.declare_dram_parameter("same_out", [JSUB, N_MASKED], F32, isOutput=True)
    dots_out = nc.declare_dram_parameter("dots_out", [1, I_PER_CORE], F32, isOutput=True)

    masked_sub = {}  # subtile index -> mask slot index
    for mi, pos in enumerate(MASK_SLOTS):
        for st in range(4):
            masked_sub[pos * 4 + st] = mi * 4 + st

    with tile.TileContext(nc) as tc:
        with (
            tc.tile_pool(name="consts", bufs=1) as consts,
            tc.tile_pool(name="simpool", bufs=3) as simpool,
            tc.tile_pool(name="small", bufs=2) as small,
            tc.tile_pool(name="plog", bufs=2, space="PSUM") as plog,
            tc.tile_pool(name="pacc", bufs=2, space="PSUM") as pacc,
        ):
            # load lhs/rhs features split in halves so compute can start early
            ftl_sb = consts.tile([D, NSUB * JSUB], BF16)
            ftr_sb = consts.tile([D, NBLOCKS * BLK], BF16)
            half_l = (NSUB * JSUB) // 2
            half_r = (NBLOCKS * BLK) // 2
            nc.sync.dma_start(out=ftl_sb[:, :half_l], in_=ftl[:, :half_l])
            nc.sync.dma_start(out=ftr_sb[:, :half_r], in_=ftr[:, :half_r])
            hx_sb = consts.tile([JSUB, NSUB * C], BF16)
            nc.sync.dma_start(out=hx_sb, in_=hx[:])
            nc.sync.dma_start(out=ftl_sb[:, half_l:], in_=ftl[:, half_l:])
            nc.sync.dma_start(out=ftr_sb[:, half_r:], in_=ftr[:, half_r:])
            a_sb = consts.tile([D, I_PER_CORE], F32)
            nc.sync.dma_start(out=a_sb, in_=a_i[:])
            b_sb = consts.tile([D, I_PER_CORE], F32)
            nc.sync.dma_start(out=b_sb, in_=b_i[:])
            msk_sb = consts.tile([JSUB, N_MASKED * BLK], BF16)
            nc.sync.dma_start(out=msk_sb, in_=msk[:])
            ones_sb = consts.tile([D, 1], F32)
            nc.vector.memset(ones_sb, 1.0)

            acc_sb = consts.tile([C, NBLOCKS * BLK], F32)
            tot_sb = consts.tile([JSUB, NSUB], F32)
            same_sb = consts.tile([JSUB, N_MASKED], F32)
            dots_sb = consts.tile([1, I_PER_CORE], F32)

            acc_ps = None
            g0 = 0
            while g0 < NSUB:
                gsz = min(ACT_GROUP, NSUB - g0)
                lg = plog.tile([JSUB, ACT_GROUP * BLK], F32, tag="lg")
                for u in range(gsz):
                    s = g0 + u
                    b = s // 4
                    nc.tensor.matmul(
                        lg[:, u * BLK:(u + 1) * BLK],
                        ftl_sb[:, s * JSUB:(s + 1) * JSUB],
                        ftr_sb[:, b * BLK:(b + 1) * BLK],
                        start=True, stop=True,
                    )
                sim = simpool.tile([JSUB, ACT_GROUP * BLK], BF16, tag="sim")
                nc.scalar.activation(
                    out=sim[:, :gsz * BLK],
                    in_=lg[:, :gsz * BLK],
                    func=mybir.ActivationFunctionType.Exp,
                    scale=SCALE,
                )
                # partition-side totals: segmented reduce over each 512 chunk
                nc.vector.reduce_sum(
                    out=tot_sb[:, g0:g0 + gsz],
                    in_=sim[:, :gsz * BLK].rearrange("p (g w) -> p g w", w=BLK),
                    axis=mybir.AxisListType.X,
                )
                for u in range(gsz):
                    s = g0 + u
                    st = s % 4
                    b = s // 4
                    if st == 0:
                        acc_ps = pacc.tile([C, BLK], F32, tag="acc")
                    nc.tensor.matmul(
                        acc_ps,
                        hx_sb[:, s * C:(s + 1) * C],
                        sim[:, u * BLK:(u + 1) * BLK],
                        start=(st == 0), stop=(st == 3),
                    )
                    if st == 3:
                        nc.vector.tensor_copy(
                            acc_sb[:, b * BLK:(b + 1) * BLK], acc_ps)
                    if s in masked_sub:
                        m = masked_sub[s]
                        trash = small.tile([JSUB, BLK], BF16, tag="trash")
                        nc.vector.tensor_mul(
                            trash,
                            sim[:, u * BLK:(u + 1) * BLK],
                            msk_sb[:, m * BLK:(m + 1) * BLK],
                        )
                        nc.vector.reduce_sum(
                            out=same_sb[:, m:m + 1],
                            in_=trash,
                            axis=mybir.AxisListType.X,
                        )
                g0 += gsz

            # f1.f2 dots
            for ib in range(I_PER_CORE // BLK):
                sl = slice(ib * BLK, (ib + 1) * BLK)
                prod = small.tile([D, BLK], F32, tag="prod")
                nc.vector.tensor_mul(prod, a_sb[:, sl], b_sb[:, sl])
                dps = pacc.tile([1, BLK], F32, tag="acc")
                nc.tensor.matmul(dps, ones_sb, prod, start=True, stop=True)
                nc.vector.tensor_copy(dots_sb[:, sl], dps)

            nc.sync.dma_start(out=acc_out[:], in_=acc_sb)
            nc.sync.dma_start(out=tot_out[:], in_=tot_sb)
            nc.sync.dma_start(out=same_out[:], in_=same_sb)
            nc.sync.dma_start(out=dots_out[:], in_=dots_sb)

    nc.compile()
    return nc


def _kernel_v2(f1, f2, label):
    if "nc2" not in _CACHE:
        _CACHE["nc2"] = _build_v2()
    nc = _CACHE["nc2"]

    feats = np.concatenate([f1, f2], axis=0)
    lab2 = np.concatenate([label, label], axis=0)
    perm = np.argsort(lab2, kind="stable")
    labs = lab2[perm]
    fsT = np.ascontiguousarray(feats[perm].T)          # [D, 2N] f32 sorted
    fsT_bf = fsT.astype(ml_dtypes.bfloat16)
    eye = np.eye(C, dtype=np.float32)
    hot = eye[labs]                                    # [2N, C] f32

    # classes must span <= 3 consecutive blocks for the mask slots to cover
    for c in range(C):
        idx = np.where(labs == c)[0]
        if idx.size and idx[-1] // BLK - idx[0] // BLK > 2:
            raise _FallbackToV1()

    f1t = np.ascontiguousarray(f1.T)
    f2t = np.ascontiguousarray(f2.T)

    in_maps = []
    per_core_blocks = []
    for k in range(NCORES):
        blocks = _core_blocks(k)
        per_core_blocks.append(blocks)
        ftl = np.empty((D, NSUB * JSUB), dtype=ml_dtypes.bfloat16)
        hx = np.empty((JSUB, NSUB * C), dtype=ml_dtypes.bfloat16)
        ftr = np.empty((D, NBLOCKS * BLK), dtype=ml_dtypes.bfloat16)
        for b, (r, c) in enumerate(blocks):
            ftr[:, b * BLK:(b + 1) * BLK] = fsT_bf[:, c * BLK:(c + 1) * BLK]
            for st in range(4):
                s = b * 4 + st
                x0 = r * BLK + st * JSUB
                ftl[:, s * JSUB:(s + 1) * JSUB] = fsT_bf[:, x0:x0 + JSUB]
                hx[:, s * C:(s + 1) * C] = hot[x0:x0 + JSUB]
        msk = np.zeros((JSUB, N_MASKED * BLK), dtype=ml_dtypes.bfloat16)
        for mi, pos in enumerate(MASK_SLOTS):
            r, c = blocks[pos]
            if r == c:
                continue
            ly = labs[c * BLK:(c + 1) * BLK]
            for st in range(4):
                lx = labs[r * BLK + st * JSUB: r * BLK + (st + 1) * JSUB]
                m = (lx[:, None] == ly[None, :])
                j = mi * 4 + st
                msk[:, j * BLK:(j + 1) * BLK] = m.astype(ml_dtypes.bfloat16)
        r0 = (k * I_PER_CORE) % N
        in_maps.append({
            "ftl": ftl, "ftr": ftr, "hx": hx, "msk": msk,
            "a_i": np.ascontiguousarray(f1t[:, r0:r0 + I_PER_CORE]),
            "b_i": np.ascontiguousarray(f2t[:, r0:r0 + I_PER_CORE]),
        })

    res = run_bass_kernel_spmd(nc, in_maps, core_ids=list(range(NCORES)))
    _CACHE["last_res"] = res

    # ---- host epilogue ----
    acc = np.zeros((TWO_N, C), dtype=np.float64)   # free-side per-class sums
    tot_p = np.zeros(TWO_N, dtype=np.float64)      # partition-side totals
    same_p = np.zeros(TWO_N, dtype=np.float64)
    dots = np.zeros(TWO_N, dtype=np.float64)
    for k in range(NCORES):
        r = res.results[k]
        blocks = per_core_blocks[k]
        acc_o = r["acc_out"].astype(np.float64)        # [C, 17*512]
        tot_o = r["tot_out"].astype(np.float64)        # [128, 68]
        same_o = r["same_out"].astype(np.float64)      # [128, 16]
        isl = slice(k * I_PER_CORE, (k + 1) * I_PER_CORE)
        dots[isl] = r["dots_out"][0].astype(np.float64)
        for b, (rr, cc) in enumerate(blocks):
            acc[cc * BLK:(cc + 1) * BLK] += acc_o[:, b * BLK:(b + 1) * BLK].T
            if rr != cc:
                for st in range(4):
                    x0 = rr * BLK + st * JSUB
                    tot_p[x0:x0 + JSUB] += tot_o[:, b * 4 + st]
        for mi, pos in enumerate(MASK_SLOTS):
            rr, cc = blocks[pos]
            if rr == cc:
                continue
            for st in range(4):
                x0 = rr * BLK + st * JSUB
                same_p[x0:x0 + JSUB] += same_o[:, mi * 4 + st]

    tot = acc.sum(axis=1) + tot_p
    same = acc[np.arange(TWO_N), labs] + same_p
    ng_sorted = tot - same
    ng = np.empty(TWO_N, dtype=np.float64)
    ng[perm] = ng_sorted
    dots[N:] = dots[:N]  # cores 4-7 computed the same dots for the f2 half
    return _finish(ng, dots, label, lab2)


class _FallbackToV1(Exception):
    pass


def _finish(ng, dots, label, lab2):
    logpos = SCALE * dots
    pos = np.exp(logpos)
    term = np.log(ng + pos) - logpos
    counts = np.bincount(label, minlength=C)
    group_size = 2.0 * counts[lab2]
    loss = np.sum(term / group_size)
    return np.float32(loss)


# ---------------------------------------------------------------------------
# non-symmetric (v1) fallback: every core computes its 1024 columns against
# all 8192 rows; per-class sums via the H-matmul alone.
# ---------------------------------------------------------------------------

V1_NB_J = TWO_N // JSUB        # 64
V1_NB_I = I_PER_CORE // BLK    # 2


def _build_v1():
    nc = bacc.Bacc("TRN2", target_bir_lowering=False, debug=False,
                   num_devices=NCORES)
    ft_all = nc.declare_dram_parameter("ft_all", [D, TWO_N], BF16, isOutput=False)
    ft_i = nc.declare_dram_parameter("ft_i", [D, I_PER_CORE], BF16, isOutput=False)
    h_all = nc.declare_dram_parameter("h_all", [JSUB, V1_NB_J * C], BF16, isOutput=False)
    a_i = nc.declare_dram_parameter("a_i", [D, I_PER_CORE], F32, isOutput=False)
    b_i = nc.declare_dram_parameter("b_i", [D, I_PER_CORE], F32, isOutput=False)
    acc_out = nc.declare_dram_parameter("acc_out", [C, I_PER_CORE], F32, isOutput=True)
    dots_out = nc.declare_dram_parameter("dots_out", [1, I_PER_CORE], F32, isOutput=True)

    stream = [(ib, js) for ib in range(V1_NB_I) for js in range(V1_NB_J)]
    n_sub = len(stream)

    with tile.TileContext(nc) as tc:
        with (
            tc.tile_pool(name="consts", bufs=1) as consts,
            tc.tile_pool(name="simpool", bufs=3) as simpool,
            tc.tile_pool(name="small", bufs=2) as small,
            tc.tile_pool(name="plog", bufs=2, space="PSUM") as plog,
            tc.tile_pool(name="pacc", bufs=2, space="PSUM") as pacc,
        ):
            ft_all_sb = consts.tile([D, TWO_N], BF16)
            nc.sync.dma_start(out=ft_all_sb, in_=ft_all[:])
            ft_i_sb = consts.tile([D, I_PER_CORE], BF16)
            nc.sync.dma_start(out=ft_i_sb, in_=ft_i[:])
            h_sb = consts.tile([JSUB, V1_NB_J * C], BF16)
            nc.sync.dma_start(out=h_sb, in_=h_all[:])
            a_sb = consts.tile([D, I_PER_CORE], F32)
            nc.sync.dma_start(out=a_sb, in_=a_i[:])
            b_sb = consts.tile([D, I_PER_CORE], F32)
            nc.sync.dma_start(out=b_sb, in_=b_i[:])
            ones_sb = consts.tile([D, 1], F32)
            nc.vector.memset(ones_sb, 1.0)

            acc_sb = consts.tile([C, I_PER_CORE], F32)
            dots_sb = consts.tile([1, I_PER_CORE], F32)

            acc_ps = None
            g0 = 0
            while g0 < n_sub:
                gsz = min(ACT_GROUP, n_sub - g0)
                lg = plog.tile([JSUB, ACT_GROUP * BLK], F32, tag="lg")
                for u in range(gsz):
                    ib, js = stream[g0 + u]
                    nc.tensor.matmul(
                        lg[:, u * BLK:(u + 1) * BLK],
                        ft_all_sb[:, js * JSUB:(js + 1) * JSUB],
                        ft_i_sb[:, ib * BLK:(ib + 1) * BLK],
                        start=True, stop=True,
                    )
                sim = simpool.tile([JSUB, ACT_GROUP * BLK], BF16, tag="sim")
                nc.scalar.activation(
                    out=sim[:, :gsz * BLK],
                    in_=lg[:, :gsz * BLK],
                    func=mybir.ActivationFunctionType.Exp,
                    scale=SCALE,
                )
                for u in range(gsz):
                    ib, js = stream[g0 + u]
                    if js == 0:
                        acc_ps = pacc.tile([C, BLK], F32, tag="acc")
                    nc.tensor.matmul(
                        acc_ps,
                        h_sb[:, js * C:(js + 1) * C],
                        sim[:, u * BLK:(u + 1) * BLK],
                        start=(js == 0), stop=(js == V1_NB_J - 1),
                    )
                    if js == V1_NB_J - 1:
                        nc.vector.tensor_copy(
                            acc_sb[:, ib * BLK:(ib + 1) * BLK], acc_ps)
                g0 += gsz

            for ib in range(V1_NB_I):
                sl = slice(ib * BLK, (ib + 1) * BLK)
                prod = small.tile([D, BLK], F32, tag="prod")
                nc.vector.tensor_mul(prod, a_sb[:, sl], b_sb[:, sl])
                dps = pacc.tile([1, BLK], F32, tag="acc")
                nc.tensor.matmul(dps, ones_sb, prod, start=True, stop=True)
                nc.vector.tensor_copy(dots_sb[:, sl], dps)

            nc.sync.dma_start(out=acc_out[:], in_=acc_sb)
            nc.sync.dma_start(out=dots_out[:], in_=dots_sb)

    nc.compile()
    return nc


def _kernel_v1(f1, f2, label):
    if "nc1" not in _CACHE:
        _CACHE["nc1"] = _build_v1()
    nc = _CACHE["nc1"]

    feats = np.concatenate([f1, f2], axis=0)
    lab2 = np.concatenate([label, label], axis=0)
    ft_bf = np.ascontiguousarray(feats.T).astype(ml_dtypes.bfloat16)
    f1t = np.ascontiguousarray(f1.T)
    f2t = np.ascontiguousarray(f2.T)

    h_pack = np.zeros((JSUB, V1_NB_J * C), dtype=ml_dtypes.bfloat16)
    eye = np.eye(C, dtype=np.float32)
    for js in range(V1_NB_J):
        rows = lab2[js * JSUB:(js + 1) * JSUB]
        h_pack[:, js * C:(js + 1) * C] = eye[rows].astype(ml_dtypes.bfloat16)

    in_maps = []
    for k in range(NCORES):
        isl = slice(k * I_PER_CORE, (k + 1) * I_PER_CORE)
        r0 = (k * I_PER_CORE) % N
        in_maps.append({
            "ft_all": ft_bf,
            "ft_i": np.ascontiguousarray(ft_bf[:, isl]),
            "h_all": h_pack,
            "a_i": np.ascontiguousarray(f1t[:, r0:r0 + I_PER_CORE]),
            "b_i": np.ascontiguousarray(f2t[:, r0:r0 + I_PER_CORE]),
        })

    res = run_bass_kernel_spmd(nc, in_maps, core_ids=list(range(NCORES)))
    _CACHE["last_res"] = res

    acc = np.zeros((C, TWO_N), dtype=np.float64)
    dots = np.zeros(TWO_N, dtype=np.float64)
    for k in range(NCORES):
        isl = slice(k * I_PER_CORE, (k + 1) * I_PER_CORE)
        acc[:, isl] = res.results[k]["acc_out"].astype(np.float64)
        dots[isl] = res.results[k]["dots_out"][0].astype(np.float64)

    tot = acc.sum(axis=0)
    same = acc[lab2, np.arange(TWO_N)]
    ng = tot - same
    return _finish(ng, dots, label, lab2)


def kernel(f1, f2, label):
    f1 = np.asarray(f1, dtype=np.float32)
    f2 = np.asarray(f2, dtype=np.float32)
    label = np.asarray(label).astype(np.int64)
    try:
        return _kernel_v2(f1, f2, label)
    except _FallbackToV1:
        return _kernel_v1(f1, f2, label)


# revision 7
# speedup vs baseline: 1.0996x; 1.0078x over previous
"""Trainium2 Bass kernel for CirculatePairConLoss.

Reference math (N=4096, D=64, C=16, T=0.05):
    feats = concat(f1, f2)                  # [2N, D]
    sim   = exp(feats @ feats.T / T)        # [2N, 2N]
    Ng_i  = sum_{j: lab_j != lab_i} sim_ij
    pos_i = exp(<f1_i, f2_i> / T)           (duplicated for both halves)
    term  = -log(pos / (Ng + pos))
    loss  = sum(term / group_size),  group_size_i = 2 * count(label == lab_i)

Device strategy (8 cores, SPMD, full I/O), symmetric version:
  Rows are sorted by label so each class is a contiguous run.  The [16x16]
  grid of 512-wide blocks of the symmetric sim matrix is covered by its
  upper triangle (136 blocks); core k owns block-rows {k, 15-k} (17 blocks
  each -- perfectly balanced).  For each block (r, c):
    * logits = Fs_r^T Fs_c on the tensor engine (bf16, K=64),
    * exp on the scalar engine (batched 1536 wide, scale=1/T folded in),
    * free side (index y in c): per-class sums via a one-hot H-matmul on
      the tensor engine, fp32 PSUM accumulation over the block,
    * partition side (index x in r): row totals via a segmented DVE
      reduce; same-class windows exist only for c in {r, r+1, r+2}
      (classes span <= 3 blocks) -- diag is covered by the H-matmul and
      the near-diag blocks get a masked fused multiply-reduce on DVE.
  Per-core outputs are tiny (class sums / totals / masked sums / f1.f2
  dots); the host does an O(2N*C) gather + log epilogue.

Every core runs the identical program; all per-core differences live in
packed input tensors (gathered slices of the sorted feature matrix, the
one-hot blocks, and the near-diagonal equality masks).
"""

import numpy as np
import ml_dtypes

import concourse.bass as bass
import concourse.tile as tile
from concourse import bacc, mybir
from concourse.bass_utils import run_bass_kernel_spmd

N = 4096
D = 64
C = 16
TWO_N = 2 * N
TEMP = 0.05
SCALE = 1.0 / TEMP  # 20.0
NCORES = 8

BLK = 512                      # block width (i and j)
JSUB = 128                     # partition-side subtile
NBLK = TWO_N // BLK            # 16
NBLOCKS = 17                   # upper-triangle blocks per core
NSUB = NBLOCKS * 4             # 68 subtiles per core
ACT_GROUP = 3                  # subtiles exp'd per ScalarE instruction
MASK_SLOTS = [13, 14, 15, 16]  # block positions that get equality masks
N_MASKED = len(MASK_SLOTS) * 4
I_PER_CORE = TWO_N // NCORES   # 1024 (for the f1.f2 dots)

BF16 = mybir.dt.bfloat16
F32 = mybir.dt.float32

_CACHE = {}


def _core_blocks(k):
    """Core k's 17 upper-triangle blocks: diag first, near-diag (mask
    candidates) in the fixed tail slots MASK_SLOTS."""
    r1, r2 = k, NBLK - 1 - k
    blocks = [(r1, c) for c in range(r1, NBLK)] + \
             [(r2, c) for c in range(r2, NBLK)]

    def prio(b):
        r, c = b
        if c == r:
            cls = 0           # diagonal
        elif c <= r + 2:
            cls = 2           # near-diagonal (needs mask)
        else:
            cls = 1           # far
        return (cls, r * NBLK + c)

    out = sorted(blocks, key=prio)
    assert len(out) == NBLOCKS
    return out


# ---------------------------------------------------------------------------
# symmetric (v2) program
# ---------------------------------------------------------------------------

def _build_v2():
    nc = bacc.Bacc("TRN2", target_bir_lowering=False, debug=False,
                   num_devices=NCORES)

    ftl = nc.declare_dram_parameter("ftl", [D, NSUB * JSUB], BF16, isOutput=False)
    ftr = nc.declare_dram_parameter("ftr", [D, NBLOCKS * BLK], BF16, isOutput=False)
    hx = nc.declare_dram_parameter("hx", [JSUB, NSUB * C], BF16, isOutput=False)
    msk = nc.declare_dram_parameter("msk", [JSUB, N_MASKED * BLK], BF16, isOutput=False)
    a_i = nc.declare_dram_parameter("a_i", [D, I_PER_CORE], F32, isOutput=False)
    b_i = nc.declare_dram_parameter("b_i", [D, I_PER_CORE], F32, isOutput=False)

    acc_out = nc.declare_dram_parameter("acc_out", [C, NBLOCKS * BLK], F32, isOutput=True)
    tot_out = nc.declare_dram_parameter("tot_out", [JSUB, NSUB], BF16, isOutput=True)
    same_out = nc.declare_dram_parameter("same_out", [JSUB, N_MASKED], BF16, isOutput=True)
    dots_out = nc.declare_dram_parameter("dots_out", [1, I_PER_CORE], F32, isOutput=True)

    masked_sub = {}  # subtile index -> mask slot index
    for mi, pos in enumerate(MASK_SLOTS):
        for st in range(4):
            masked_sub[pos * 4 + st] = mi * 4 + st

    with tile.TileContext(nc) as tc:
        with (
            tc.tile_pool(name="consts", bufs=1) as consts,
            tc.tile_pool(name="simpool", bufs=3) as simpool,
            tc.tile_pool(name="small", bufs=2) as small,
            tc.tile_pool(name="plog", bufs=2, space="PSUM") as plog,
            tc.tile_pool(name="pacc", bufs=2, space="PSUM") as pacc,
        ):
            # load lhs/rhs features split in halves so compute can start early
            ftl_sb = consts.tile([D, NSUB * JSUB], BF16)
            ftr_sb = consts.tile([D, NBLOCKS * BLK], BF16)
            half_l = (NSUB * JSUB) // 2
            half_r = (NBLOCKS * BLK) // 2
            nc.sync.dma_start(out=ftl_sb[:, :half_l], in_=ftl[:, :half_l])
            nc.sync.dma_start(out=ftr_sb[:, :half_r], in_=ftr[:, :half_r])
            hx_sb = consts.tile([JSUB, NSUB * C], BF16)
            nc.sync.dma_start(out=hx_sb, in_=hx[:])
            nc.sync.dma_start(out=ftl_sb[:, half_l:], in_=ftl[:, half_l:])
            nc.sync.dma_start(out=ftr_sb[:, half_r:], in_=ftr[:, half_r:])
            a_sb = consts.tile([D, I_PER_CORE], F32)
            nc.sync.dma_start(out=a_sb, in_=a_i[:])
            b_sb = consts.tile([D, I_PER_CORE], F32)
            nc.sync.dma_start(out=b_sb, in_=b_i[:])
            msk_sb = consts.tile([JSUB, N_MASKED * BLK], BF16)
            nc.sync.dma_start(out=msk_sb, in_=msk[:])
            ones_sb = consts.tile([D, 1], F32)
            nc.vector.memset(ones_sb, 1.0)

            acc_sb = consts.tile([C, NBLOCKS * BLK], F32)
            tot_sb = consts.tile([JSUB, NSUB], BF16)
            same_sb = consts.tile([JSUB, N_MASKED], BF16)
            dots_sb = consts.tile([1, I_PER_CORE], F32)

            acc_ps = None
            g0 = 0
            while g0 < NSUB:
                gsz = min(ACT_GROUP, NSUB - g0)
                lg = plog.tile([JSUB, ACT_GROUP * BLK], F32, tag="lg")
                for u in range(gsz):
                    s = g0 + u
                    b = s // 4
                    nc.tensor.matmul(
                        lg[:, u * BLK:(u + 1) * BLK],
                        ftl_sb[:, s * JSUB:(s + 1) * JSUB],
                        ftr_sb[:, b * BLK:(b + 1) * BLK],
                        start=True, stop=True,
                    )
                sim = simpool.tile([JSUB, ACT_GROUP * BLK], BF16, tag="sim")
                nc.scalar.activation(
                    out=sim[:, :gsz * BLK],
                    in_=lg[:, :gsz * BLK],
                    func=mybir.ActivationFunctionType.Exp,
                    scale=SCALE,
                )
                # partition-side totals: segmented reduce over each 512 chunk
                with nc.allow_low_precision(reason="bf16 partials; host sums in f64"):
                    nc.vector.reduce_sum(
                        out=tot_sb[:, g0:g0 + gsz],
                        in_=sim[:, :gsz * BLK].rearrange("p (g w) -> p g w", w=BLK),
                        axis=mybir.AxisListType.X,
                    )
                for u in range(gsz):
                    s = g0 + u
                    st = s % 4
                    b = s // 4
                    if st == 0:
                        acc_ps = pacc.tile([C, BLK], F32, tag="acc")
                    nc.tensor.matmul(
                        acc_ps,
                        hx_sb[:, s * C:(s + 1) * C],
                        sim[:, u * BLK:(u + 1) * BLK],
                        start=(st == 0), stop=(st == 3),
                    )
                    if st == 3:
                        nc.vector.tensor_copy(
                            acc_sb[:, b * BLK:(b + 1) * BLK], acc_ps)
                    if s in masked_sub:
                        m = masked_sub[s]
                        trash = small.tile([JSUB, BLK], BF16, tag="trash")
                        nc.gpsimd.tensor_mul(
                            trash,
                            sim[:, u * BLK:(u + 1) * BLK],
                            msk_sb[:, m * BLK:(m + 1) * BLK],
                        )
                        with nc.allow_low_precision(reason="bf16 partials"):
                            nc.vector.reduce_sum(
                                out=same_sb[:, m:m + 1],
                                in_=trash,
                                axis=mybir.AxisListType.X,
                            )
                g0 += gsz

            # f1.f2 dots
            for ib in range(I_PER_CORE // BLK):
                sl = slice(ib * BLK, (ib + 1) * BLK)
                prod = small.tile([D, BLK], F32, tag="prod")
                nc.vector.tensor_mul(prod, a_sb[:, sl], b_sb[:, sl])
                dps = pacc.tile([1, BLK], F32, tag="acc")
                nc.tensor.matmul(dps, ones_sb, prod, start=True, stop=True)
                nc.vector.tensor_copy(dots_sb[:, sl], dps)

            nc.sync.dma_start(out=acc_out[:], in_=acc_sb)
            nc.sync.dma_start(out=tot_out[:], in_=tot_sb)
            nc.sync.dma_start(out=same_out[:], in_=same_sb)
            nc.sync.dma_start(out=dots_out[:], in_=dots_sb)

    nc.compile()
    return nc


def _kernel_v2(f1, f2, label):
    if "nc2" not in _CACHE:
        _CACHE["nc2"] = _build_v2()
    nc = _CACHE["nc2"]

    feats = np.concatenate([f1, f2], axis=0)
    lab2 = np.concatenate([label, label], axis=0)
    perm = np.argsort(lab2, kind="stable")
    labs = lab2[perm]
    fsT = np.ascontiguousarray(feats[perm].T)          # [D, 2N] f32 sorted
    fsT_bf = fsT.astype(ml_dtypes.bfloat16)
    eye = np.eye(C, dtype=np.float32)
    hot = eye[labs]                                    # [2N, C] f32

    # classes must span <= 3 consecutive blocks for the mask slots to cover
    for c in range(C):
        idx = np.where(labs == c)[0]
        if idx.size and idx[-1] // BLK - idx[0] // BLK > 2:
            raise _FallbackToV1()

    f1t = np.ascontiguousarray(f1.T)
    f2t = np.ascontiguousarray(f2.T)

    in_maps = []
    per_core_blocks = []
    for k in range(NCORES):
        blocks = _core_blocks(k)
        per_core_blocks.append(blocks)
        ftl = np.empty((D, NSUB * JSUB), dtype=ml_dtypes.bfloat16)
        hx = np.empty((JSUB, NSUB * C), dtype=ml_dtypes.bfloat16)
        ftr = np.empty((D, NBLOCKS * BLK), dtype=ml_dtypes.bfloat16)
        for b, (r, c) in enumerate(blocks):
            ftr[:, b * BLK:(b + 1) * BLK] = fsT_bf[:, c * BLK:(c + 1) * BLK]
            for st in range(4):
                s = b * 4 + st
                x0 = r * BLK + st * JSUB
                ftl[:, s * JSUB:(s + 1) * JSUB] = fsT_bf[:, x0:x0 + JSUB]
                hx[:, s * C:(s + 1) * C] = hot[x0:x0 + JSUB]
        msk = np.zeros((JSUB, N_MASKED * BLK), dtype=ml_dtypes.bfloat16)
        for mi, pos in enumerate(MASK_SLOTS):
            r, c = blocks[pos]
            if r == c:
                continue
            ly = labs[c * BLK:(c + 1) * BLK]
            for st in range(4):
                lx = labs[r * BLK + st * JSUB: r * BLK + (st + 1) * JSUB]
                m = (lx[:, None] == ly[None, :])
                j = mi * 4 + st
                msk[:, j * BLK:(j + 1) * BLK] = m.astype(ml_dtypes.bfloat16)
        r0 = (k * I_PER_CORE) % N
        in_maps.append({
            "ftl": ftl, "ftr": ftr, "hx": hx, "msk": msk,
            "a_i": np.ascontiguousarray(f1t[:, r0:r0 + I_PER_CORE]),
            "b_i": np.ascontiguousarray(f2t[:, r0:r0 + I_PER_CORE]),
        })

    res = run_bass_kernel_spmd(nc, in_maps, core_ids=list(range(NCORES)))
    _CACHE["last_res"] = res

    # ---- host epilogue ----
    acc = np.zeros((TWO_N, C), dtype=np.float64)   # free-side per-class sums
    tot_p = np.zeros(TWO_N, dtype=np.float64)      # partition-side totals
    same_p = np.zeros(TWO_N, dtype=np.float64)
    dots = np.zeros(TWO_N, dtype=np.float64)
    for k in range(NCORES):
        r = res.results[k]
        blocks = per_core_blocks[k]
        acc_o = r["acc_out"].astype(np.float64)        # [C, 17*512]
        tot_o = r["tot_out"].astype(np.float64)        # [128, 68]
        same_o = r["same_out"].astype(np.float64)      # [128, 16]
        isl = slice(k * I_PER_CORE, (k + 1) * I_PER_CORE)
        dots[isl] = r["dots_out"][0].astype(np.float64)
        for b, (rr, cc) in enumerate(blocks):
            acc[cc * BLK:(cc + 1) * BLK] += acc_o[:, b * BLK:(b + 1) * BLK].T
            if rr != cc:
                for st in range(4):
                    x0 = rr * BLK + st * JSUB
                    tot_p[x0:x0 + JSUB] += tot_o[:, b * 4 + st]
        for mi, pos in enumerate(MASK_SLOTS):
            rr, cc = blocks[pos]
            if rr == cc:
                continue
            for st in range(4):
                x0 = rr * BLK + st * JSUB
                same_p[x0:x0 + JSUB] += same_o[:, mi * 4 + st]

    tot = acc.sum(axis=1) + tot_p
    same = acc[np.arange(TWO_N), labs] + same_p
    ng_sorted = tot - same
    ng = np.empty(TWO_N, dtype=np.float64)
    ng[perm] = ng_sorted
    dots[N:] = dots[:N]  # cores 4-7 computed the same dots for the f2 half
    return _finish(ng, dots, label, lab2)


class _FallbackToV1(Exception):
    pass


def _finish(ng, dots, label, lab2):
    logpos = SCALE * dots
    pos = np.exp(logpos)
    term = np.log(ng + pos) - logpos
    counts = np.bincount(label, minlength=C)
    group_size = 2.0 * counts[lab2]
    loss = np.sum(term / group_size)
    return np.float32(loss)


# ---------------------------------------------------------------------------
# non-symmetric (v1) fallback: every core computes its 1024 columns against
# all 8192 rows; per-class sums via the H-matmul alone.
# ---------------------------------------------------------------------------

V1_NB_J = TWO_N // JSUB        # 64
V1_NB_I = I_PER_CORE // BLK    # 2


def _build_v1():
    nc = bacc.Bacc("TRN2", target_bir_lowering=False, debug=False,
                   num_devices=NCORES)
    ft_all = nc.declare_dram_parameter("ft_all", [D, TWO_N], BF16, isOutput=False)
    ft_i = nc.declare_dram_parameter("ft_i", [D, I_PER_CORE], BF16, isOutput=False)
    h_all = nc.declare_dram_parameter("h_all", [JSUB, V1_NB_J * C], BF16, isOutput=False)
    a_i = nc.declare_dram_parameter("a_i", [D, I_PER_CORE], F32, isOutput=False)
    b_i = nc.declare_dram_parameter("b_i", [D, I_PER_CORE], F32, isOutput=False)
    acc_out = nc.declare_dram_parameter("acc_out", [C, I_PER_CORE], F32, isOutput=True)
    dots_out = nc.declare_dram_parameter("dots_out", [1, I_PER_CORE], F32, isOutput=True)

    stream = [(ib, js) for ib in range(V1_NB_I) for js in range(V1_NB_J)]
    n_sub = len(stream)

    with tile.TileContext(nc) as tc:
        with (
            tc.tile_pool(name="consts", bufs=1) as consts,
            tc.tile_pool(name="simpool", bufs=3) as simpool,
            tc.tile_pool(name="small", bufs=2) as small,
            tc.tile_pool(name="plog", bufs=2, space="PSUM") as plog,
            tc.tile_pool(name="pacc", bufs=2, space="PSUM") as pacc,
        ):
            ft_all_sb = consts.tile([D, TWO_N], BF16)
            nc.sync.dma_start(out=ft_all_sb, in_=ft_all[:])
            ft_i_sb = consts.tile([D, I_PER_CORE], BF16)
            nc.sync.dma_start(out=ft_i_sb, in_=ft_i[:])
            h_sb = consts.tile([JSUB, V1_NB_J * C], BF16)
            nc.sync.dma_start(out=h_sb, in_=h_all[:])
            a_sb = consts.tile([D, I_PER_CORE], F32)
            nc.sync.dma_start(out=a_sb, in_=a_i[:])
            b_sb = consts.tile([D, I_PER_CORE], F32)
            nc.sync.dma_start(out=b_sb, in_=b_i[:])
            ones_sb = consts.tile([D, 1], F32)
            nc.vector.memset(ones_sb, 1.0)

            acc_sb = consts.tile([C, I_PER_CORE], F32)
            dots_sb = consts.tile([1, I_PER_CORE], F32)

            acc_ps = None
            g0 = 0
            while g0 < n_sub:
                gsz = min(ACT_GROUP, n_sub - g0)
                lg = plog.tile([JSUB, ACT_GROUP * BLK], F32, tag="lg")
                for u in range(gsz):
                    ib, js = stream[g0 + u]
                    nc.tensor.matmul(
                        lg[:, u * BLK:(u + 1) * BLK],
                        ft_all_sb[:, js * JSUB:(js + 1) * JSUB],
                        ft_i_sb[:, ib * BLK:(ib + 1) * BLK],
                        start=True, stop=True,
                    )
                sim = simpool.tile([JSUB, ACT_GROUP * BLK], BF16, tag="sim")
                nc.scalar.activation(
                    out=sim[:, :gsz * BLK],
                    in_=lg[:, :gsz * BLK],
                    func=mybir.ActivationFunctionType.Exp,
                    scale=SCALE,
                )
                for u in range(gsz):
                    ib, js = stream[g0 + u]
                    if js == 0:
                        acc_ps = pacc.tile([C, BLK], F32, tag="acc")
                    nc.tensor.matmul(
                        acc_ps,
                        h_sb[:, js * C:(js + 1) * C],
                        sim[:, u * BLK:(u + 1) * BLK],
                        start=(js == 0), stop=(js == V1_NB_J - 1),
                    )
                    if js == V1_NB_J - 1:
                        nc.vector.tensor_copy(
                            acc_sb[:, ib * BLK:(ib + 1) * BLK], acc_ps)
                g0 += gsz

            for ib in range(V1_NB_I):
                sl = slice(ib * BLK, (ib + 1) * BLK)
                prod = small.tile([D, BLK], F32, tag="prod")
                nc.vector.tensor_mul(prod, a_sb[:, sl], b_sb[:, sl])
                dps = pacc.tile([1, BLK], F32, tag="acc")
                nc.tensor.matmul(dps, ones_sb, prod, start=True, stop=True)
                nc.vector.tensor_copy(dots_sb[:, sl], dps)

            nc.sync.dma_start(out=acc_out[:], in_=acc_sb)
            nc.sync.dma_start(out=dots_out[:], in_=dots_sb)

    nc.compile()
    return nc


def _kernel_v1(f1, f2, label):
    if "nc1" not in _CACHE:
        _CACHE["nc1"] = _build_v1()
    nc = _CACHE["nc1"]

    feats = np.concatenate([f1, f2], axis=0)
    lab2 = np.concatenate([label, label], axis=0)
    ft_bf = np.ascontiguousarray(feats.T).astype(ml_dtypes.bfloat16)
    f1t = np.ascontiguousarray(f1.T)
    f2t = np.ascontiguousarray(f2.T)

    h_pack = np.zeros((JSUB, V1_NB_J * C), dtype=ml_dtypes.bfloat16)
    eye = np.eye(C, dtype=np.float32)
    for js in range(V1_NB_J):
        rows = lab2[js * JSUB:(js + 1) * JSUB]
        h_pack[:, js * C:(js + 1) * C] = eye[rows].astype(ml_dtypes.bfloat16)

    in_maps = []
    for k in range(NCORES):
        isl = slice(k * I_PER_CORE, (k + 1) * I_PER_CORE)
        r0 = (k * I_PER_CORE) % N
        in_maps.append({
            "ft_all": ft_bf,
            "ft_i": np.ascontiguousarray(ft_bf[:, isl]),
            "h_all": h_pack,
            "a_i": np.ascontiguousarray(f1t[:, r0:r0 + I_PER_CORE]),
            "b_i": np.ascontiguousarray(f2t[:, r0:r0 + I_PER_CORE]),
        })

    res = run_bass_kernel_spmd(nc, in_maps, core_ids=list(range(NCORES)))
    _CACHE["last_res"] = res

    acc = np.zeros((C, TWO_N), dtype=np.float64)
    dots = np.zeros(TWO_N, dtype=np.float64)
    for k in range(NCORES):
        isl = slice(k * I_PER_CORE, (k + 1) * I_PER_CORE)
        acc[:, isl] = res.results[k]["acc_out"].astype(np.float64)
        dots[isl] = res.results[k]["dots_out"][0].astype(np.float64)

    tot = acc.sum(axis=0)
    same = acc[lab2, np.arange(TWO_N)]
    ng = tot - same
    return _finish(ng, dots, label, lab2)


def kernel(f1, f2, label):
    f1 = np.asarray(f1, dtype=np.float32)
    f2 = np.asarray(f2, dtype=np.float32)
    label = np.asarray(label).astype(np.int64)
    try:
        return _kernel_v2(f1, f2, label)
    except _FallbackToV1:
        return _kernel_v1(f1, f2, label)


# revision 8
# speedup vs baseline: 1.1287x; 1.0265x over previous
"""Trainium2 Bass kernel for CirculatePairConLoss.

Reference math (N=4096, D=64, C=16, T=0.05):
    feats = concat(f1, f2)                  # [2N, D]
    sim   = exp(feats @ feats.T / T)        # [2N, 2N]
    Ng_i  = sum_{j: lab_j != lab_i} sim_ij
    pos_i = exp(<f1_i, f2_i> / T)           (duplicated for both halves)
    term  = -log(pos / (Ng + pos))
    loss  = sum(term / group_size),  group_size_i = 2 * count(label == lab_i)

Device strategy (8 cores, SPMD, full I/O), symmetric version:
  Rows are sorted by label so each class is a contiguous run.  The [16x16]
  grid of 512-wide blocks of the symmetric sim matrix is covered by its
  upper triangle (136 blocks); core k owns block-rows {k, 15-k} (17 blocks
  each -- perfectly balanced).  For each block (r, c):
    * logits = Fs_r^T Fs_c on the tensor engine (bf16, K=64),
    * exp on the scalar engine (batched 1536 wide, scale=1/T folded in),
    * free side (index y in c): per-class sums via a one-hot H-matmul on
      the tensor engine, fp32 PSUM accumulation over the block,
    * partition side (index x in r): row totals via a segmented DVE
      reduce; same-class windows exist only for c in {r, r+1, r+2}
      (classes span <= 3 blocks) -- diag is covered by the H-matmul and
      the near-diag blocks get a masked fused multiply-reduce on DVE.
  Per-core outputs are tiny (class sums / totals / masked sums / f1.f2
  dots); the host does an O(2N*C) gather + log epilogue.

Every core runs the identical program; all per-core differences live in
packed input tensors (gathered slices of the sorted feature matrix, the
one-hot blocks, and the near-diagonal equality masks).
"""

import numpy as np
import ml_dtypes

import concourse.bass as bass
import concourse.tile as tile
from concourse import bacc, mybir
from concourse.bass_utils import run_bass_kernel_spmd

N = 4096
D = 64
C = 16
TWO_N = 2 * N
TEMP = 0.05
SCALE = 1.0 / TEMP  # 20.0
NCORES = 8

BLK = 512                      # block width (i and j)
JSUB = 128                     # partition-side subtile
NBLK = TWO_N // BLK            # 16
NBLOCKS = 17                   # upper-triangle blocks per core
NSUB = NBLOCKS * 4             # 68 subtiles per core
ACT_GROUP = 3                  # subtiles exp'd per ScalarE instruction
NGROUPS = (NSUB + ACT_GROUP - 1) // ACT_GROUP  # 23
MASK_SLOTS = [13, 14, 15, 16]  # block positions that get equality masks
N_MASKED = len(MASK_SLOTS) * 4
I_PER_CORE = TWO_N // NCORES   # 1024 (for the f1.f2 dots)

BF16 = mybir.dt.bfloat16
F32 = mybir.dt.float32

_CACHE = {}


def _core_blocks(k):
    """Core k's 17 upper-triangle blocks: diag first, near-diag (mask
    candidates) in the fixed tail slots MASK_SLOTS."""
    r1, r2 = k, NBLK - 1 - k
    blocks = [(r1, c) for c in range(r1, NBLK)] + \
             [(r2, c) for c in range(r2, NBLK)]

    def prio(b):
        r, c = b
        if c == r:
            cls = 0           # diagonal
        elif c <= r + 2:
            cls = 2           # near-diagonal (needs mask)
        else:
            cls = 1           # far
        return (cls, r * NBLK + c)

    out = sorted(blocks, key=prio)
    assert len(out) == NBLOCKS
    return out


# ---------------------------------------------------------------------------
# symmetric (v2) program
# ---------------------------------------------------------------------------

def _build_v2():
    nc = bacc.Bacc("TRN2", target_bir_lowering=False, debug=False,
                   num_devices=NCORES)

    ftl = nc.declare_dram_parameter("ftl", [D, NSUB * JSUB], BF16, isOutput=False)
    ftr = nc.declare_dram_parameter("ftr", [D, NBLOCKS * BLK], BF16, isOutput=False)
    hx = nc.declare_dram_parameter("hx", [JSUB, NSUB * C], BF16, isOutput=False)
    msk = nc.declare_dram_parameter("msk", [JSUB, N_MASKED * BLK], BF16, isOutput=False)
    a_i = nc.declare_dram_parameter("a_i", [D, I_PER_CORE], F32, isOutput=False)
    b_i = nc.declare_dram_parameter("b_i", [D, I_PER_CORE], F32, isOutput=False)

    acc_out = nc.declare_dram_parameter("acc_out", [C, NBLOCKS * BLK], F32, isOutput=True)
    tot_out = nc.declare_dram_parameter("tot_out", [JSUB, NGROUPS * 4], BF16, isOutput=True)
    same_out = nc.declare_dram_parameter("same_out", [JSUB, N_MASKED], BF16, isOutput=True)
    dots_out = nc.declare_dram_parameter("dots_out", [1, I_PER_CORE], F32, isOutput=True)

    masked_sub = {}  # subtile index -> mask slot index
    for mi, pos in enumerate(MASK_SLOTS):
        for st in range(4):
            masked_sub[pos * 4 + st] = mi * 4 + st

    with tile.TileContext(nc) as tc:
        with (
            tc.tile_pool(name="consts", bufs=1) as consts,
            tc.tile_pool(name="simpool", bufs=3) as simpool,
            tc.tile_pool(name="small", bufs=2) as small,
            tc.tile_pool(name="plog", bufs=2, space="PSUM") as plog,
            tc.tile_pool(name="pacc", bufs=2, space="PSUM") as pacc,
        ):
            # load lhs/rhs features split in halves so compute can start early
            ftl_sb = consts.tile([D, NSUB * JSUB], BF16)
            ftr_sb = consts.tile([D, NBLOCKS * BLK], BF16)
            half_l = (NSUB * JSUB) // 2
            half_r = (NBLOCKS * BLK) // 2
            nc.sync.dma_start(out=ftl_sb[:, :half_l], in_=ftl[:, :half_l])
            nc.sync.dma_start(out=ftr_sb[:, :half_r], in_=ftr[:, :half_r])
            hx_sb = consts.tile([JSUB, NSUB * C], BF16)
            nc.sync.dma_start(out=hx_sb, in_=hx[:])
            nc.sync.dma_start(out=ftl_sb[:, half_l:], in_=ftl[:, half_l:])
            nc.sync.dma_start(out=ftr_sb[:, half_r:], in_=ftr[:, half_r:])
            a_sb = consts.tile([D, I_PER_CORE], F32)
            nc.sync.dma_start(out=a_sb, in_=a_i[:])
            b_sb = consts.tile([D, I_PER_CORE], F32)
            nc.sync.dma_start(out=b_sb, in_=b_i[:])
            msk_sb = consts.tile([JSUB, N_MASKED * BLK], BF16)
            nc.sync.dma_start(out=msk_sb, in_=msk[:])
            ones_sb = consts.tile([D, 1], F32)
            nc.vector.memset(ones_sb, 1.0)

            acc_sb = consts.tile([C, NBLOCKS * BLK], F32)
            tot_sb = consts.tile([JSUB, NGROUPS * 4], BF16)
            same_sb = consts.tile([JSUB, N_MASKED], BF16)
            dots_sb = consts.tile([1, I_PER_CORE], F32)

            acc_ps = None
            mtrash = None
            g0 = 0
            while g0 < NSUB:
                gsz = min(ACT_GROUP, NSUB - g0)
                lg = plog.tile([JSUB, ACT_GROUP * BLK], F32, tag="lg")
                for u in range(gsz):
                    s = g0 + u
                    b = s // 4
                    nc.tensor.matmul(
                        lg[:, u * BLK:(u + 1) * BLK],
                        ftl_sb[:, s * JSUB:(s + 1) * JSUB],
                        ftr_sb[:, b * BLK:(b + 1) * BLK],
                        start=True, stop=True,
                    )
                sim = simpool.tile([JSUB, ACT_GROUP * BLK], BF16, tag="sim")
                nc.scalar.activation(
                    out=sim[:, :gsz * BLK],
                    in_=lg[:, :gsz * BLK],
                    func=mybir.ActivationFunctionType.Exp,
                    scale=SCALE,
                )
                # partition-side totals: segmented reduce over each 512 chunk
                gslot = (g0 // ACT_GROUP) * 4
                with nc.allow_low_precision(reason="bf16 partials; host sums in f64"):
                    nc.vector.reduce_sum(
                        out=tot_sb[:, gslot:gslot + gsz],
                        in_=sim[:, :gsz * BLK].rearrange("p (g w) -> p g w", w=BLK),
                        axis=mybir.AxisListType.X,
                    )
                for u in range(gsz):
                    s = g0 + u
                    st = s % 4
                    b = s // 4
                    if st == 0:
                        acc_ps = pacc.tile([C, BLK], F32, tag="acc")
                    nc.tensor.matmul(
                        acc_ps,
                        hx_sb[:, s * C:(s + 1) * C],
                        sim[:, u * BLK:(u + 1) * BLK],
                        start=(st == 0), stop=(st == 3),
                    )
                    if st == 3:
                        nc.vector.tensor_copy(
                            acc_sb[:, b * BLK:(b + 1) * BLK], acc_ps)
                    if s in masked_sub:
                        m = masked_sub[s]
                        if m % 4 == 0:
                            mtrash = small.tile([JSUB, 4 * BLK], BF16, tag="mtrash")
                        nc.gpsimd.tensor_mul(
                            mtrash[:, (m % 4) * BLK:(m % 4 + 1) * BLK],
                            sim[:, u * BLK:(u + 1) * BLK],
                            msk_sb[:, m * BLK:(m + 1) * BLK],
                        )
                        if m % 4 == 3:
                            mi4 = m - 3
                            with nc.allow_low_precision(reason="bf16 partials"):
                                nc.vector.reduce_sum(
                                    out=same_sb[:, mi4:mi4 + 4],
                                    in_=mtrash.rearrange("p (g w) -> p g w", w=BLK),
                                    axis=mybir.AxisListType.X,
                                )
                g0 += gsz

            # f1.f2 dots
            for ib in range(I_PER_CORE // BLK):
                sl = slice(ib * BLK, (ib + 1) * BLK)
                prod = small.tile([D, BLK], F32, tag="prod")
                nc.vector.tensor_mul(prod, a_sb[:, sl], b_sb[:, sl])
                dps = pacc.tile([1, BLK], F32, tag="acc")
                nc.tensor.matmul(dps, ones_sb, prod, start=True, stop=True)
                nc.vector.tensor_copy(dots_sb[:, sl], dps)

            nc.sync.dma_start(out=acc_out[:], in_=acc_sb)
            nc.sync.dma_start(out=tot_out[:], in_=tot_sb)
            nc.sync.dma_start(out=same_out[:], in_=same_sb)
            nc.sync.dma_start(out=dots_out[:], in_=dots_sb)

    nc.compile()
    return nc


def _kernel_v2(f1, f2, label):
    if "nc2" not in _CACHE:
        _CACHE["nc2"] = _build_v2()
    nc = _CACHE["nc2"]

    feats = np.concatenate([f1, f2], axis=0)
    lab2 = np.concatenate([label, label], axis=0)
    perm = np.argsort(lab2, kind="stable")
    labs = lab2[perm]
    fsT = np.ascontiguousarray(feats[perm].T)          # [D, 2N] f32 sorted
    fsT_bf = fsT.astype(ml_dtypes.bfloat16)
    eye = np.eye(C, dtype=np.float32)
    hot = eye[labs]                                    # [2N, C] f32

    # classes must span <= 3 consecutive blocks for the mask slots to cover
    for c in range(C):
        idx = np.where(labs == c)[0]
        if idx.size and idx[-1] // BLK - idx[0] // BLK > 2:
            raise _FallbackToV1()

    f1t = np.ascontiguousarray(f1.T)
    f2t = np.ascontiguousarray(f2.T)

    in_maps = []
    per_core_blocks = []
    for k in range(NCORES):
        blocks = _core_blocks(k)
        per_core_blocks.append(blocks)
        ftl = np.empty((D, NSUB * JSUB), dtype=ml_dtypes.bfloat16)
        hx = np.empty((JSUB, NSUB * C), dtype=ml_dtypes.bfloat16)
        ftr = np.empty((D, NBLOCKS * BLK), dtype=ml_dtypes.bfloat16)
        for b, (r, c) in enumerate(blocks):
            ftr[:, b * BLK:(b + 1) * BLK] = fsT_bf[:, c * BLK:(c + 1) * BLK]
            for st in range(4):
                s = b * 4 + st
                x0 = r * BLK + st * JSUB
                ftl[:, s * JSUB:(s + 1) * JSUB] = fsT_bf[:, x0:x0 + JSUB]
                hx[:, s * C:(s + 1) * C] = hot[x0:x0 + JSUB]
        msk = np.zeros((JSUB, N_MASKED * BLK), dtype=ml_dtypes.bfloat16)
        for mi, pos in enumerate(MASK_SLOTS):
            r, c = blocks[pos]
            if r == c:
                continue
            ly = labs[c * BLK:(c + 1) * BLK]
            for st in range(4):
                lx = labs[r * BLK + st * JSUB: r * BLK + (st + 1) * JSUB]
                m = (lx[:, None] == ly[None, :])
                j = mi * 4 + st
                msk[:, j * BLK:(j + 1) * BLK] = m.astype(ml_dtypes.bfloat16)
        r0 = (k * I_PER_CORE) % N
        in_maps.append({
            "ftl": ftl, "ftr": ftr, "hx": hx, "msk": msk,
            "a_i": np.ascontiguousarray(f1t[:, r0:r0 + I_PER_CORE]),
            "b_i": np.ascontiguousarray(f2t[:, r0:r0 + I_PER_CORE]),
        })

    res = run_bass_kernel_spmd(nc, in_maps, core_ids=list(range(NCORES)))
    _CACHE["last_res"] = res

    # ---- host epilogue ----
    acc = np.zeros((TWO_N, C), dtype=np.float64)   # free-side per-class sums
    tot_p = np.zeros(TWO_N, dtype=np.float64)      # partition-side totals
    same_p = np.zeros(TWO_N, dtype=np.float64)
    dots = np.zeros(TWO_N, dtype=np.float64)
    for k in range(NCORES):
        r = res.results[k]
        blocks = per_core_blocks[k]
        acc_o = r["acc_out"].astype(np.float64)        # [C, 17*512]
        tot_g = r["tot_out"].astype(np.float64)        # [128, NGROUPS*4]
        tot_o = np.empty((JSUB, NSUB))
        for s in range(NSUB):
            tot_o[:, s] = tot_g[:, (s // ACT_GROUP) * 4 + s % ACT_GROUP]
        same_o = r["same_out"].astype(np.float64)      # [128, 16]
        isl = slice(k * I_PER_CORE, (k + 1) * I_PER_CORE)
        dots[isl] = r["dots_out"][0].astype(np.float64)
        for b, (rr, cc) in enumerate(blocks):
            acc[cc * BLK:(cc + 1) * BLK] += acc_o[:, b * BLK:(b + 1) * BLK].T
            if rr != cc:
                for st in range(4):
                    x0 = rr * BLK + st * JSUB
                    tot_p[x0:x0 + JSUB] += tot_o[:, b * 4 + st]
        for mi, pos in enumerate(MASK_SLOTS):
            rr, cc = blocks[pos]
            if rr == cc:
                continue
            for st in range(4):
                x0 = rr * BLK + st * JSUB
                same_p[x0:x0 + JSUB] += same_o[:, mi * 4 + st]

    tot = acc.sum(axis=1) + tot_p
    same = acc[np.arange(TWO_N), labs] + same_p
    ng_sorted = tot - same
    ng = np.empty(TWO_N, dtype=np.float64)
    ng[perm] = ng_sorted
    dots[N:] = dots[:N]  # cores 4-7 computed the same dots for the f2 half
    return _finish(ng, dots, label, lab2)


class _FallbackToV1(Exception):
    pass


def _finish(ng, dots, label, lab2):
    logpos = SCALE * dots
    pos = np.exp(logpos)
    term = np.log(ng + pos) - logpos
    counts = np.bincount(label, minlength=C)
    group_size = 2.0 * counts[lab2]
    loss = np.sum(term / group_size)
    return np.float32(loss)


# ---------------------------------------------------------------------------
# non-symmetric (v1) fallback: every core computes its 1024 columns against
# all 8192 rows; per-class sums via the H-matmul alone.
# ---------------------------------------------------------------------------

V1_NB_J = TWO_N // JSUB        # 64
V1_NB_I = I_PER_CORE // BLK    # 2


def _build_v1():
    nc = bacc.Bacc("TRN2", target_bir_lowering=False, debug=False,
                   num_devices=NCORES)
    ft_all = nc.declare_dram_parameter("ft_all", [D, TWO_N], BF16, isOutput=False)
    ft_i = nc.declare_dram_parameter("ft_i", [D, I_PER_CORE], BF16, isOutput=False)
    h_all = nc.declare_dram_parameter("h_all", [JSUB, V1_NB_J * C], BF16, isOutput=False)
    a_i = nc.declare_dram_parameter("a_i", [D, I_PER_CORE], F32, isOutput=False)
    b_i = nc.declare_dram_parameter("b_i", [D, I_PER_CORE], F32, isOutput=False)
    acc_out = nc.declare_dram_parameter("acc_out", [C, I_PER_CORE], F32, isOutput=True)
    dots_out = nc.declare_dram_parameter("dots_out", [1, I_PER_CORE], F32, isOutput=True)

    stream = [(ib, js) for ib in range(V1_NB_I) for js in range(V1_NB_J)]
    n_sub = len(stream)

    with tile.TileContext(nc) as tc:
        with (
            tc.tile_pool(name="consts", bufs=1) as consts,
            tc.tile_pool(name="simpool", bufs=3) as simpool,
            tc.tile_pool(name="small", bufs=2) as small,
            tc.tile_pool(name="plog", bufs=2, space="PSUM") as plog,
            tc.tile_pool(name="pacc", bufs=2, space="PSUM") as pacc,
        ):
            ft_all_sb = consts.tile([D, TWO_N], BF16)
            nc.sync.dma_start(out=ft_all_sb, in_=ft_all[:])
            ft_i_sb = consts.tile([D, I_PER_CORE], BF16)
            nc.sync.dma_start(out=ft_i_sb, in_=ft_i[:])
            h_sb = consts.tile([JSUB, V1_NB_J * C], BF16)
            nc.sync.dma_start(out=h_sb, in_=h_all[:])
            a_sb = consts.tile([D, I_PER_CORE], F32)
            nc.sync.dma_start(out=a_sb, in_=a_i[:])
            b_sb = consts.tile([D, I_PER_CORE], F32)
            nc.sync.dma_start(out=b_sb, in_=b_i[:])
            ones_sb = consts.tile([D, 1], F32)
            nc.vector.memset(ones_sb, 1.0)

            acc_sb = consts.tile([C, I_PER_CORE], F32)
            dots_sb = consts.tile([1, I_PER_CORE], F32)

            acc_ps = None
            g0 = 0
            while g0 < n_sub:
                gsz = min(ACT_GROUP, n_sub - g0)
                lg = plog.tile([JSUB, ACT_GROUP * BLK], F32, tag="lg")
                for u in range(gsz):
                    ib, js = stream[g0 + u]
                    nc.tensor.matmul(
                        lg[:, u * BLK:(u + 1) * BLK],
                        ft_all_sb[:, js * JSUB:(js + 1) * JSUB],
                        ft_i_sb[:, ib * BLK:(ib + 1) * BLK],
                        start=True, stop=True,
                    )
                sim = simpool.tile([JSUB, ACT_GROUP * BLK], BF16, tag="sim")
                nc.scalar.activation(
                    out=sim[:, :gsz * BLK],
                    in_=lg[:, :gsz * BLK],
                    func=mybir.ActivationFunctionType.Exp,
                    scale=SCALE,
                )
                for u in range(gsz):
                    ib, js = stream[g0 + u]
                    if js == 0:
                        acc_ps = pacc.tile([C, BLK], F32, tag="acc")
                    nc.tensor.matmul(
                        acc_ps,
                        h_sb[:, js * C:(js + 1) * C],
                        sim[:, u * BLK:(u + 1) * BLK],
                        start=(js == 0), stop=(js == V1_NB_J - 1),
                    )
                    if js == V1_NB_J - 1:
                        nc.vector.tensor_copy(
                            acc_sb[:, ib * BLK:(ib + 1) * BLK], acc_ps)
                g0 += gsz

            for ib in range(V1_NB_I):
                sl = slice(ib * BLK, (ib + 1) * BLK)
                prod = small.tile([D, BLK], F32, tag="prod")
                nc.vector.tensor_mul(prod, a_sb[:, sl], b_sb[:, sl])
                dps = pacc.tile([1, BLK], F32, tag="acc")
                nc.tensor.matmul(dps, ones_sb, prod, start=True, stop=True)
                nc.vector.tensor_copy(dots_sb[:, sl], dps)

            nc.sync.dma_start(out=acc_out[:], in_=acc_sb)
            nc.sync.dma_start(out=dots_out[:], in_=dots_sb)

    nc.compile()
    return nc


def _kernel_v1(f1, f2, label):
    if "nc1" not in _CACHE:
        _CACHE["nc1"] = _build_v1()
    nc = _CACHE["nc1"]

    feats = np.concatenate([f1, f2], axis=0)
    lab2 = np.concatenate([label, label], axis=0)
    ft_bf = np.ascontiguousarray(feats.T).astype(ml_dtypes.bfloat16)
    f1t = np.ascontiguousarray(f1.T)
    f2t = np.ascontiguousarray(f2.T)

    h_pack = np.zeros((JSUB, V1_NB_J * C), dtype=ml_dtypes.bfloat16)
    eye = np.eye(C, dtype=np.float32)
    for js in range(V1_NB_J):
        rows = lab2[js * JSUB:(js + 1) * JSUB]
        h_pack[:, js * C:(js + 1) * C] = eye[rows].astype(ml_dtypes.bfloat16)

    in_maps = []
    for k in range(NCORES):
        isl = slice(k * I_PER_CORE, (k + 1) * I_PER_CORE)
        r0 = (k * I_PER_CORE) % N
        in_maps.append({
            "ft_all": ft_bf,
            "ft_i": np.ascontiguousarray(ft_bf[:, isl]),
            "h_all": h_pack,
            "a_i": np.ascontiguousarray(f1t[:, r0:r0 + I_PER_CORE]),
            "b_i": np.ascontiguousarray(f2t[:, r0:r0 + I_PER_CORE]),
        })

    res = run_bass_kernel_spmd(nc, in_maps, core_ids=list(range(NCORES)))
    _CACHE["last_res"] = res

    acc = np.zeros((C, TWO_N), dtype=np.float64)
    dots = np.zeros(TWO_N, dtype=np.float64)
    for k in range(NCORES):
        isl = slice(k * I_PER_CORE, (k + 1) * I_PER_CORE)
        acc[:, isl] = res.results[k]["acc_out"].astype(np.float64)
        dots[isl] = res.results[k]["dots_out"][0].astype(np.float64)

    tot = acc.sum(axis=0)
    same = acc[lab2, np.arange(TWO_N)]
    ng = tot - same
    return _finish(ng, dots, label, lab2)


def kernel(f1, f2, label):
    f1 = np.asarray(f1, dtype=np.float32)
    f2 = np.asarray(f2, dtype=np.float32)
    label = np.asarray(label).astype(np.int64)
    try:
        return _kernel_v2(f1, f2, label)
    except _FallbackToV1:
        return _kernel_v1(f1, f2, label)
